# revision 54
# baseline (speedup 1.0000x reference)
"""Causal multi-head attention (B=4, S=2048, D=1024, H=16, Hd=64) on 8 TRN2
NeuronCores.

Sharding: tensor-parallel over heads. Core c owns heads [2c, 2c+1]:
  - Wq/Wk/Wv column-sharded (rows of the [out,in] weight): each core projects
    x -> qT/kT/vT [128, S] (2 heads x 64, head-dim-major).
  - Attention per (b, h) computed entirely on-core, scoresT layout
    [keys, queries] so softmax normalization folds into matmuls.
  - Wo row-sharded: each core emits a partial [B,S,D] output; host sums the
    8 partials.

Optimizations vs the straightforward schedule (435us -> ~290us):
  - Causal trim: the 4 diagonal key-strips of each query chunk compute only
    queries >= strip start (widths 512/384/256/128 instead of 4x512), cutting
    scores+AV matmul columns and exp volume ~15%.
  - One [128,2(head),512] PSUM tile per key strip: a single exp instruction
    covers both heads, and ps_p's two buffers give a true one-group score
    lookahead ahead of the exp-dependent AV matmuls.
  - Single [128,128] lower-triangle mask applied to the leading 128 columns
    of each diagonal strip (one strided multiply covering both heads).
  - Self-weaving pipeline: each attention chunk interleaves, between its
    score/AV groups, the projection of the NEXT chunk of the same batch and
    the output projection of the previous chunk, so the PE never drains --
    including during the last batch.
  - Free Z broadcast: vext carries 64 ones-columns, so the AV matmul
    itself replicates the softmax denominator across pav rows 64-127
    (matmul cost is output-width-based); normalize is then just
    copy+reciprocal+multiply on the DVE -- no GpSimd broadcast.
  - Batched DMA: x loaded in [128,4,512] tiles (2 DMAs/chunk), output stored
    in [128,1024] tiles (1 DMA per 128 tokens).

Numerics: matmul operands in bf16 (fp32 PSUM accumulation), softmax without
max-subtraction (scores are bounded ~|10| for this data distribution), causal
mask applied post-exp as a {0,1} multiply.
"""

import os
import numpy as np
import ml_dtypes
from contextlib import ExitStack

import concourse.bass as bass
import concourse.tile as tile
from concourse import bacc, mybir
from concourse.bass_utils import run_bass_kernel_spmd
from concourse.masks import make_identity

F32 = mybir.dt.float32
BF16 = mybir.dt.bfloat16
NPBF16 = ml_dtypes.bfloat16

B, S, D = 4, 2048, 1024
H, HD = 16, 64
NCORES = 8
HPC = H // NCORES          # heads per core
DH = HPC * HD              # local head dim (128)
TC = 512                   # token chunk for projections / query chunk
KS = 128                   # key strip

last_exec_time_ns = None   # set by kernel() when BASS_TRACE=1


def emit(tc_ctx: tile.TileContext, ctx: ExitStack, aps: dict, b_count: int, seq: int):
    """Emit the per-core program. aps: xt [b,D,seq] bf16, wq/wk/wv [D,DH] bf16,
    wo [DH,D] bf16, mask [128, 128] bf16, out [b,seq,D] bf16."""
    nc = tc_ctx.nc
    tc = tc_ctx
    KC = D // 128            # contraction chunks for projections
    NTC = seq // TC          # token chunks
    NQC = seq // TC          # query chunks
    NKS = seq // KS          # key strips

    xt, wq, wk, wv, wo, mask, out = (
        aps["xt"], aps["wq"], aps["wk"], aps["wv"], aps["wo"], aps["mask"], aps["out"]
    )

    wpool = ctx.enter_context(tc.tile_pool(name="wpool", bufs=1))
    xpool = ctx.enter_context(tc.tile_pool(name="xpool", bufs=4))
    qkpool = ctx.enter_context(tc.tile_pool(name="qkpool", bufs=4))
    vpool = ctx.enter_context(tc.tile_pool(name="vpool", bufs=2))
    ppool = ctx.enter_context(tc.tile_pool(name="ppool", bufs=6))
    avpool = ctx.enter_context(tc.tile_pool(name="avpool", bufs=4))
    smalls = ctx.enter_context(tc.tile_pool(name="smalls", bufs=8))

    ps_scr = ctx.enter_context(tc.tile_pool(name="ps_scr", bufs=2, space="PSUM"))
    ps_p = ctx.enter_context(tc.tile_pool(name="ps_p", bufs=2, space="PSUM"))
    ps_av = ctx.enter_context(tc.tile_pool(name="ps_av", bufs=2, space="PSUM"))

    # --- constants / weights ---
    w_sb = {}
    for name, ap in (("wq", wq), ("wk", wk), ("wv", wv)):
        t = wpool.tile([128, KC, DH], BF16, tag=name, name=f"w_{name}")
        nc.sync.dma_start(out=t, in_=ap.rearrange("(kc p) m -> p kc m", p=128))
        w_sb[name] = t
    wo_sb = wpool.tile([128, D], BF16)
    nc.sync.dma_start(out=wo_sb, in_=wo)
    mask_sb = wpool.tile([128, 128], BF16)
    nc.sync.dma_start(out=mask_sb, in_=mask)

    ident_f = wpool.tile([128, 64], F32)
    make_identity(nc, ident_f[0:64, :])
    make_identity(nc, ident_f[64:128, :])
    ident = wpool.tile([128, 64], BF16)
    nc.vector.tensor_copy(ident, ident_f)

    ones_f = wpool.tile([128, 64], F32)
    nc.vector.memset(ones_f, 1.0)
    ones_r = wpool.tile([128, 64], BF16)
    nc.vector.tensor_copy(ones_r, ones_f)

    qTs, kTs, vexts, avTs = {}, {}, {}, {}

    def proj_fillers(b, tcc):
        """Projection work for one 512-token chunk of batch b, as a list of
        closures so it can be interleaved between attention groups."""
        def dmas():
            xt_src = xt[b].rearrange("(kh kc p) t -> p kh kc t", p=128, kh=2)
            for kh in range(2):  # two 4-chunk DMAs instead of eight 1-chunk
                t = xpool.tile([128, KC // 2, TC], BF16, tag="xt",
                               name=f"xt_{b}_{tcc}_{kh}", bufs=6)
                nc.sync.dma_start(out=t,
                                  in_=xt_src[:, kh, :, tcc * TC:(tcc + 1) * TC])
                xt_ts[(b, tcc, kh)] = t

        def wgroup(name):
            def go():
                dst = {"wq": qTs[b], "wk": kTs[b], "wv": vexts[(b, "vT")]}[name]
                ps = ps_scr.tile([128, TC], F32, tag="scr", name=f"ps_{name}")
                for kc in range(KC):
                    nc.tensor.matmul(ps, w_sb[name][:, kc, :],
                                     xt_ts[(b, tcc, kc // 4)][:, kc % 4, :],
                                     start=(kc == 0), stop=(kc == KC - 1))
                nc.vector.tensor_copy(dst[:, tcc * TC:(tcc + 1) * TC], ps)
            return go

        def vtrans(h):
            def go():
                vext = vexts[b]
                vT = vexts[(b, "vT")]
                tr4 = ps_scr.tile([128, 4, 64], BF16, tag="scr", name="tr4")
                for i in range(4):
                    ks = tcc * 4 + i
                    nc.tensor.transpose(
                        tr4[:, i, :], vT[h * 64:(h + 1) * 64, ks * 128:(ks + 1) * 128],
                        ident[h * 64:(h + 1) * 64, :])
                nc.vector.tensor_copy(vext[:, h, tcc * 4:(tcc + 1) * 4, 0:64], tr4)
                nc.vector.tensor_copy(
                    vext[:, h, tcc * 4:(tcc + 1) * 4, 64:128],
                    ones_r.rearrange("p (o m) -> p o m", o=1)
                    .to_broadcast([128, 4, 64]))
            return go

        return [dmas, wgroup("wq"), wgroup("wk"), wgroup("wv"),
                vtrans(0), vtrans(1)]

    def alloc_batch(b):
        qTs[b] = qkpool.tile([128, seq], BF16, tag="qT", name=f"qT{b}")
        kTs[b] = qkpool.tile([128, seq], BF16, tag="kT", name=f"kT{b}")
        vexts[(b, "vT")] = vpool.tile([128, seq], BF16, tag="vT", name=f"vT{b}")
        vexts[b] = vpool.tile([128, HPC, NKS, 128], BF16, tag="vext",
                              name=f"vext{b}", bufs=3)

    def emit_attn_qc(b, qc, fillers):
        """One query-chunk of attention for batch b, causal-trimmed.

        One group per key strip, both heads sharing a [128, 2(head), 512]
        PSUM tile: 4*qc full strips plus 4 diagonal strips trimmed to
        queries >= strip start (widths 512/384/256/128). Scores of group g+1
        and interleaved filler work are emitted before the exp-dependent AV
        matmuls of group g so the PE never stalls on the ACT.
        """
        qT, kT, vext = qTs[b], kTs[b], vexts[b]
        avT = avTs[b]
        d0 = 4 * qc
        # one group per key strip; BOTH heads share a [128, 2(head), 512]
        # PSUM tile, so ps_p's bufs=2 gives a true one-group lookahead.
        # group := (st, q_off, width, diag?)
        groups = [(st, 0, TC, False) for st in range(d0)]
        for i in range(4):
            groups.append((d0 + i, 128 * i, TC - 128 * i, True))
        last_av_st = d0 + 3

        pav = {h: ps_av.tile([128, TC], F32, tag="av", name=f"pav{h}")
               for h in range(HPC)}

        def emit_scores(grp):
            st, qo, w, diag = grp
            pp = ps_p.tile([128, 2, TC], F32, tag="pp", name="pp")
            for h in range(HPC):
                nc.tensor.matmul(
                    pp[:, h, 0:w],
                    kT[h * 64:(h + 1) * 64, st * KS:(st + 1) * KS],
                    qT[h * 64:(h + 1) * 64, qc * TC + qo:qc * TC + qo + w],
                    start=True, stop=True)
            return pp

        def emit_expmaskav(grp, pp):
            st, qo, w, diag = grp
            p_sb = ppool.tile([128, 2, TC], BF16, tag="p", name="p_sb")
            if w == TC:
                nc.scalar.activation(p_sb.rearrange("p a b -> p (a b)"),
                                     pp.rearrange("p a b -> p (a b)"),
                                     mybir.ActivationFunctionType.Exp)
            else:
                nc.scalar.activation(p_sb[:, :, 0:w], pp[:, :, 0:w],
                                     mybir.ActivationFunctionType.Exp)
            if diag:  # triangle mask on the leading 128 cols of both heads
                nc.vector.tensor_mul(
                    p_sb[:, :, 0:128], p_sb[:, :, 0:128],
                    mask_sb.rearrange("p (o m) -> p o m", o=1)
                    .to_broadcast([128, 2, 128]))
            for h in range(HPC):
                nc.tensor.matmul(pav[h][:, qo:qo + w], vext[:, h, st, :],
                                 p_sb[:, h, 0:w],
                                 start=(st == 0), stop=(st == last_av_st))

        fi = 0
        def fill(n):
            nonlocal fi
            for _ in range(n):
                if fi < len(fillers):
                    fillers[fi]()
                    fi += 1

        prev = None
        ngaps = len(groups)
        for gi, grp in enumerate(groups):
            pp = emit_scores(grp)
            # distribute fillers evenly across the group gaps
            want = -((len(fillers) - fi) // -(ngaps - gi))
            fill(want)
            if prev is not None:
                emit_expmaskav(prev[0], prev[1])
            prev = (grp, pp)
        emit_expmaskav(prev[0], prev[1])

        for h in range(HPC):
            z64 = smalls.tile([64, TC], F32, tag="z64")
            nc.vector.tensor_copy(z64, pav[h][64:128, :])
            rz = smalls.tile([64, TC], F32, tag="rz")
            nc.vector.reciprocal_approx_fast(rz, z64)
            with nc.allow_low_precision(reason="attn weights tolerate bf16"):
                nc.vector.tensor_mul(avT[h * 64:(h + 1) * 64, qc * TC:(qc + 1) * TC],
                                     pav[h][0:64, :], rz)
        fill(len(fillers) - fi)

    def outproj_fillers(b, qc):
        """Output projection of chunk qc of batch b as filler closures, one
        per (token-128 block, 512-col half) for fine interleaving."""
        o_sbs = {}

        def piece(t4, n2):
            def go():
                avT = avTs[b]
                t16 = qc * (TC // 128) + t4
                if n2 == 0:
                    o_sbs[t4] = smalls.tile([128, 2, TC], BF16, tag="o",
                                            name="o_sb")
                o_sb = o_sbs[t4]
                po = ps_scr.tile([128, TC], F32, tag="scr", name="po")
                nc.tensor.matmul(po, avT[:, t16 * 128:(t16 + 1) * 128],
                                 wo_sb[:, n2 * TC:(n2 + 1) * TC],
                                 start=True, stop=True)
                nc.vector.tensor_copy(o_sb[:, n2, :], po)
                if n2 == D // TC - 1:
                    nc.sync.dma_start(
                        out=out[b, t16 * 128:(t16 + 1) * 128, :],
                        in_=o_sb.rearrange("p a b -> p (a b)"))
            return go
        return [piece(t4, n2) for t4 in range(TC // 128)
                for n2 in range(D // TC)]

    # Self-weaving pipeline: attention(b, qc) interleaves, at group
    # granularity, the projection of chunk qc+1 of the SAME batch (chunk 0 of
    # b+1 at the last qc) plus the output projection of the previous chunk.
    # Every qc of every batch -- including the last -- has independent PE
    # filler, so the tensor engine never drains while the ACT runs exp.
    xt_ts = {}
    alloc_batch(0)
    avTs[0] = avpool.tile([128, seq], BF16, tag="avT", name="avT0")
    for f in proj_fillers(0, 0):
        f()
    for b in range(b_count):
        if b + 1 < b_count:
            alloc_batch(b + 1)
            avTs[b + 1] = avpool.tile([128, seq], BF16, tag="avT",
                                      name=f"avT{b + 1}")
        last = b == b_count - 1
        for qc in range(NQC):
            pf, of = [], []
            if qc + 1 < NQC:
                pf = proj_fillers(b, qc + 1)
            elif b + 1 < b_count:
                pf = proj_fillers(b + 1, 0)
            if last:
                # no woven projections after this batch: delay the output
                # projections one extra chunk so the final (filler-starved)
                # query chunks still have independent PE work
                if qc == 0 and b > 0:
                    of = outproj_fillers(b - 1, NQC - 1)
                elif qc == 2:
                    of = outproj_fillers(b, 0)
                elif qc == 3:
                    of = outproj_fillers(b, 1) + outproj_fillers(b, 2)
            else:
                if qc > 0:
                    of = outproj_fillers(b, qc - 1)
                elif b > 0:
                    of = outproj_fillers(b - 1, NQC - 1)
            emit_attn_qc(b, qc, pf + of)
    for f in outproj_fillers(b_count - 1, NQC - 1):
        f()


def host_inputs(x, Wq, Wk, Wv, Wo, core, xt_bf=None):
    """Build the per-core input map."""
    hs = slice(core * DH, (core + 1) * DH)
    if xt_bf is None:
        xt_bf = np.ascontiguousarray(np.transpose(x, (0, 2, 1))).astype(NPBF16)
    wq = np.ascontiguousarray((Wq[hs, :] * np.float32(1.0 / np.sqrt(HD))).T).astype(NPBF16)
    wk = np.ascontiguousarray(Wk[hs, :].T).astype(NPBF16)
    wv = np.ascontiguousarray(Wv[hs, :].T).astype(NPBF16)
    wo = np.ascontiguousarray(Wo[:, hs].T).astype(NPBF16)
    # [128,128] lower-triangle (keys on partitions): mask[k, q] = (q >= k)
    mask = (np.arange(128)[None, :] >= np.arange(128)[:, None]).astype(NPBF16)
    return {"xt": xt_bf, "wq": wq, "wk": wk, "wv": wv, "wo": wo, "mask": mask}


def build_program(b_count=B, seq=S):
    nc = bacc.Bacc("TRN2", target_bir_lowering=False, debug=False,
                   num_devices=NCORES)
    aps = {
        "xt": nc.dram_tensor("xt", [b_count, D, seq], BF16, kind="ExternalInput").ap(),
        "wq": nc.dram_tensor("wq", [D, DH], BF16, kind="ExternalInput").ap(),
        "wk": nc.dram_tensor("wk", [D, DH], BF16, kind="ExternalInput").ap(),
        "wv": nc.dram_tensor("wv", [D, DH], BF16, kind="ExternalInput").ap(),
        "wo": nc.dram_tensor("wo", [DH, D], BF16, kind="ExternalInput").ap(),
        "mask": nc.dram_tensor("mask", [128, 128], BF16, kind="ExternalInput").ap(),
        "out": nc.dram_tensor("out", [b_count, seq, D], BF16, kind="ExternalOutput").ap(),
    }
    with tile.TileContext(nc) as tcx:
        with ExitStack() as ctx:
            emit(tcx, ctx, aps, b_count, seq)
    nc.finalize()
    return nc


def _ensure_ntff_hook():
    """Register the ctypes NTFF profile hook when the image lacks
    antenv.axon_hooks (needed only for trace=True)."""
    import sys, types
    try:
        import antenv.axon_hooks  # noqa: F401
        return
    except ImportError:
        pass
    try:
        import antenv
        from trn_agent_boot.trn_boot import _ntff_profile_via_ctypes
        hook = _ntff_profile_via_ctypes("/opt/axon/libaxon_pjrt.so")
        mod = types.ModuleType("antenv.axon_hooks")
        mod.get_axon_ntff_profile_hook = lambda: hook
        mod.set_axon_ntff_profile_hook = lambda h: None
        sys.modules["antenv.axon_hooks"] = mod
        antenv.axon_hooks = mod
    except Exception:
        pass


def kernel(x, Wq, Wk, Wv, Wo):
    global last_exec_time_ns
    x = np.asarray(x, dtype=np.float32)
    Wq = np.asarray(Wq, dtype=np.float32)
    Wk = np.asarray(Wk, dtype=np.float32)
    Wv = np.asarray(Wv, dtype=np.float32)
    Wo = np.asarray(Wo, dtype=np.float32)

    nc = build_program(B, S)
    xt_bf = np.ascontiguousarray(np.transpose(x, (0, 2, 1))).astype(NPBF16)
    in_maps = [host_inputs(x, Wq, Wk, Wv, Wo, c, xt_bf=xt_bf) for c in range(NCORES)]
    trace = bool(os.environ.get("BASS_TRACE"))
    if trace:
        _ensure_ntff_hook()
    res = run_bass_kernel_spmd(nc, in_maps, list(range(NCORES)), trace=trace)
    last_exec_time_ns = res.exec_time_ns
    parts = [res.results[c]["out"] for c in range(NCORES)]
    acc = parts[0].astype(np.float32)
    for p in parts[1:]:
        acc = acc + p
    return acc
